# revision 2
# baseline (speedup 1.0000x reference)
"""Contrastive (InfoNCE-style symmetric) loss on 8 trn2 NeuronCores.

Reference math (B=4096, D=1024, fp32):
    xn = x / max(||x_i||, eps);  yn = y / max(||y_j||, eps)
    S[i,j] = xn_i . yn_j ;  E = exp(S/tau)
    extra = B*eps + eps
    row_denom_i = sum_j E[i,j] + extra ; col_denom_j = sum_i E[i,j] + extra
    loss = -1/(2B) * ( 2*sum_i S_ii/tau - sum_i ln(row_denom_i)
                       - sum_j ln(col_denom_j) )

Sharding: batch dim of x is split across the 8 cores (512 rows each); every
core holds the full y (transposed). Each core computes its [4096, 512] block
of S^T on TensorE (j on partitions, local i on free).

v1 structure (vs the AllGather baseline): every core computes ALL 4096 y
norms locally (DVE squares + ones-matmul partition reduction), so no
collective sits on the exp critical path. x is pre-scaled by 1/||x_i|| in
bf16 before the main matmuls, so the ScalarE ACT drains PSUM directly:
eb = exp(S^T_block * ry_scl) with the per-partition scale 1/(tau*||y_j||),
emitting per-block column partials via accum_out. Row denominators
accumulate on TensorE as ones^T @ E, interleaved into the stream with a
2-block lag. A single merged AllReduce at the end carries the 4096 column
partials + the diag/row scalar terms (collectives here are latency-bound
~10us each, so one beats three). yT is DMA'd in 128KB column-chunk-major
pieces so norm matmuls trickle in during the load and keep the PE's HAM
clock-gate warm. A tiny AllGather issued first absorbs the collective
entry barrier early.
"""
import numpy as np
import ml_dtypes

import concourse.bacc as bacc
import concourse.mybir as mybir
import concourse.tile as tile
from concourse.bass_utils import run_bass_kernel_spmd

AF = mybir.ActivationFunctionType
ALU = mybir.AluOpType
BF16 = mybir.dt.bfloat16
F32 = mybir.dt.float32

B = 4096
D = 1024
N_CORES = 8
BL = B // N_CORES          # 512 local x rows
TAU = 0.07
EPS = 1e-6
EXTRA = B * EPS + EPS
COEF = -1.0 / (2.0 * B)

ND = D // 128              # 8 contraction chunks
NJB = B // 128             # 32 j-blocks (PSUM partition dim)
NCH = B // 512             # 8 column chunks of 512 js
N_WARM = 8                 # dummy matmuls to warm the PE clock gate
RS_LAG = 2                 # row-sum matmul lag (blocks) behind the exp stage

_cache: dict = {}


def _build():
    nc = bacc.Bacc("TRN2", target_bir_lowering=False, debug=False,
                   num_devices=N_CORES)

    xT = nc.dram_tensor("xT", [D, BL], BF16, kind="ExternalInput")
    yT = nc.dram_tensor("yT", [D, B], BF16, kind="ExternalInput")
    yTo = nc.dram_tensor("yTown", [D, BL], BF16, kind="ExternalInput")
    loss_out = nc.dram_tensor("loss", [1, 1], F32, kind="ExternalOutput")

    rg = [list(range(N_CORES))]

    with tile.TileContext(nc) as tc:
        with (
            tc.tile_pool(name="res", bufs=1) as res,
            tc.tile_pool(name="tmp", bufs=3) as tmp,
            tc.tile_pool(name="eblk", bufs=8) as epool,
            tc.tile_pool(name="pg", bufs=3, space="PSUM") as pg,
            tc.tile_pool(name="pny", bufs=2, space="PSUM") as pny,
            tc.tile_pool(name="pa", bufs=1, space="PSUM") as pa,
            tc.tile_pool(name="pw", bufs=1, space="PSUM") as pw,
            tc.tile_pool(name="prow", bufs=1, space="PSUM") as prow,
            tc.tile_pool(name="dram", bufs=1, space="DRAM") as dr,
        ):
            # ---- tiny AllGather first: absorb the collective entry barrier
            dumm_in = dr.tile([8], F32, name="dumm_in")
            dumm_out = dr.tile([64], F32, name="dumm_out")
            zz = res.tile([1, 8], F32, name="zz")
            nc.vector.memset(zz[:], 0.0)
            nc.gpsimd.dma_start(dumm_in[:], zz[:])
            nc.gpsimd.collective_compute(
                "AllGather", ALU.bypass, replica_groups=rg,
                ins=[dumm_in.opt()], outs=[dumm_out.opt()])

            # ---- PE warm-up: dummy matmuls while input DMAs fly ----
            wsrc = res.tile([128, 512], BF16, name="wsrc")
            nc.vector.memset(wsrc[:], 0.125)
            wp = pw.tile([128, 512], F32, tag="pw", name="wp")
            for _ in range(N_WARM):
                nc.tensor.matmul(wp[:], wsrc[:, 0:128], wsrc[:],
                                 start=True, stop=True, skip_group_check=True)

            # ---- input DMAs (priority order: xT, yT g0 c-major, g1, yTown)
            xts = []
            for d in range(ND):
                t = res.tile([128, BL], BF16, tag=f"xt{d}", name=f"xt{d}")
                nc.sync.dma_start(t[:], xT[d * 128:(d + 1) * 128, :])
                xts.append(t)
            yts = {}
            for g2 in range(2):
                for d in range(ND):
                    yts[(g2, d)] = res.tile([128, 2048], BF16,
                                            tag=f"yt{g2}_{d}",
                                            name=f"yt{g2}_{d}")
            for g2 in range(2):
                for cl in range(4):       # column chunk within group
                    for d in range(ND):
                        c = g2 * 4 + cl
                        nc.sync.dma_start(
                            yts[(g2, d)][:, cl * 512:(cl + 1) * 512],
                            yT[d * 128:(d + 1) * 128,
                               c * 512:(c + 1) * 512])
            ytos = []
            for d in range(ND):
                t = res.tile([128, BL], BF16, tag=f"yo{d}", name=f"yo{d}")
                nc.sync.dma_start(t[:], yTo[d * 128:(d + 1) * 128, :])
                ytos.append(t)

            ones_bf = res.tile([128, 1], BF16, name="ones_bf")
            nc.vector.memset(ones_bf[:], 1.0)
            ones_f = res.tile([128, 1], F32, name="ones_f")
            nc.vector.memset(ones_f[:], 1.0)

            # ---- ||x||^2 -> rx chain (gates the xn pre-scale) ----
            p_nx = pa.tile([1, 512], F32, tag="pa", name="p_nx")
            for d in range(ND):
                sq = tmp.tile([128, 512], BF16, tag="sq", name="sq")
                nc.vector.tensor_mul(sq[:], xts[d][:], xts[d][:])
                nc.tensor.matmul(p_nx[:], ones_bf[:], sq[:],
                                 start=(d == 0), stop=(d == ND - 1),
                                 skip_group_check=True)
            nx = tmp.tile([1, 512], F32, tag="v", name="nx")
            nc.scalar.activation(nx[:], p_nx[:], AF.Sqrt)
            nxm = tmp.tile([1, 512], F32, tag="v", name="nxm")
            nc.vector.tensor_scalar_max(nxm[:], nx[:], EPS)
            rx = res.tile([1, 512], F32, name="rx")
            nc.vector.reciprocal(rx[:], nxm[:])
            rx_d = dr.tile([BL], F32, name="rx_d")
            nc.gpsimd.dma_start(rx_d[:], rx[:])
            rx_b = res.tile([128, 512], F32, name="rx_b")
            nc.gpsimd.dma_start(
                rx_b[:],
                rx_d[:].rearrange("(o a) -> o a", o=1).broadcast_to([128, BL]))
            # xn = x * (1/||x||)  (bf16, feeds the main matmul stream)
            xns = []
            for d in range(ND):
                t = res.tile([128, BL], BF16, tag=f"xn{d}", name=f"xn{d}")
                nc.vector.tensor_mul(t[:], xts[d][:], rx_b[:])
                xns.append(t)

            # ---- all-y norms, chunk by chunk: ry_scl[p, jb] = 1/(tau*||y_j||)
            # for j = jb*128 + p.  PE: ones^T @ (y char chunk)^2, PSUM-acc over d.
            ry_scl = res.tile([128, 32], F32, name="ry_scl")
            rys_d = dr.tile([B], F32, name="rys_d")

            def emit_ynorm_chunk(c):
                g2, cl = c // 4, c % 4
                p_ny = pny.tile([1, 512], F32, tag="pny", name=f"p_ny{c}")
                for d in range(ND):
                    sqy = tmp.tile([128, 512], BF16, tag="sq", name=f"sqy{c}_{d}")
                    nc.vector.tensor_mul(
                        sqy[:],
                        yts[(g2, d)][:, cl * 512:(cl + 1) * 512],
                        yts[(g2, d)][:, cl * 512:(cl + 1) * 512])
                    nc.tensor.matmul(p_ny[:], ones_bf[:], sqy[:],
                                     start=(d == 0), stop=(d == ND - 1),
                                     skip_group_check=True)
                return p_ny

            def emit_ynorm_chain(c, p_ny):
                ny = tmp.tile([1, 512], F32, tag="v", name=f"ny{c}")
                nc.scalar.activation(ny[:], p_ny[:], AF.Sqrt)
                nym = tmp.tile([1, 512], F32, tag="v", name=f"nym{c}")
                nc.vector.tensor_scalar_max(nym[:], ny[:], EPS)
                ryc = tmp.tile([1, 512], F32, tag="v", name=f"ryc{c}")
                nc.vector.reciprocal(ryc[:], nym[:])
                rysc = tmp.tile([1, 512], F32, tag="v2", name=f"rysc{c}")
                nc.vector.tensor_scalar_mul(rysc[:], ryc[:], 1.0 / TAU)
                nc.gpsimd.dma_start(rys_d[c * 512:(c + 1) * 512], rysc[:])
                nc.gpsimd.dma_start(
                    ry_scl[:, 4 * c:4 * c + 4],
                    rys_d[512 * c:512 * (c + 1)].rearrange(
                        "(a b) -> b a", b=128))

            pnys = {}
            for c in range(4):            # group g2=0 norms (pre-stream)
                pnys[c] = emit_ynorm_chunk(c)
                if c >= 1:
                    emit_ynorm_chain(c - 1, pnys.pop(c - 1))
            emit_ynorm_chain(3, pnys.pop(3))

            # ---- main loop ----
            colpart = res.tile([128, 32], F32, name="colpart")
            dk_rk = res.tile([1, 8], F32, name="dk_rk")
            nc.vector.memset(dk_rk[:], 0.0)
            e_blks = {}
            p_row = prow.tile([1, 512], F32, tag="prow", name="p_row")

            def emit_rowmm(jb):
                nc.tensor.matmul(p_row[:], ones_bf[:], e_blks.pop(jb)[:],
                                 start=(jb == 0), stop=(jb == NJB - 1),
                                 skip_group_check=True)

            def emit_main_block(jb):
                g2, joff = jb // 16, (jb % 16) * 128
                pgt = pg.tile([128, 512], F32, tag="pg", name="pg")
                for d in range(ND):
                    nc.tensor.matmul(
                        pgt[:],
                        yts[(g2, d)][:, joff:joff + 128],
                        xns[d][:],
                        start=(d == 0), stop=(d == ND - 1),
                        skip_group_check=True)
                eb = epool.tile([128, 512], BF16, tag="eb", name="eb")
                nc.scalar.activation(eb[:], pgt[:], AF.Exp,
                                     scale=ry_scl[:, jb:jb + 1],
                                     accum_out=colpart[:, jb:jb + 1])
                e_blks[jb] = eb
                if jb >= RS_LAG:
                    emit_rowmm(jb - RS_LAG)

            for jb in range(16):
                emit_main_block(jb)

            # group g2=1 norms (data arrived during jb0..15)
            for c in range(4, 8):
                pnys[c] = emit_ynorm_chunk(c)
                if c >= 5:
                    emit_ynorm_chain(c - 1, pnys.pop(c - 1))
            emit_ynorm_chain(7, pnys.pop(7))

            # ---- y_own norm chain + diag-dot (feeds dk_rk[0] only) ----
            p_nyo = pa.tile([1, 512], F32, tag="pa", name="p_nyo")
            for d in range(ND):
                sq2 = tmp.tile([128, 512], BF16, tag="sq", name=f"sqo{d}")
                nc.vector.tensor_mul(sq2[:], ytos[d][:], ytos[d][:])
                nc.tensor.matmul(p_nyo[:], ones_bf[:], sq2[:],
                                 start=(d == 0), stop=(d == ND - 1),
                                 skip_group_check=True)
            nyo = tmp.tile([1, 512], F32, tag="v", name="nyo")
            nc.scalar.activation(nyo[:], p_nyo[:], AF.Sqrt)
            nyom = tmp.tile([1, 512], F32, tag="v", name="nyom")
            nc.vector.tensor_scalar_max(nyom[:], nyo[:], EPS)
            ryo = res.tile([1, 512], F32, name="ryo")
            nc.vector.reciprocal(ryo[:], nyom[:])

            p_dd = pa.tile([1, 512], F32, tag="pa", name="p_dd")
            for d in range(ND):
                prd = tmp.tile([128, 512], BF16, tag="sq", name=f"prd{d}")
                nc.vector.tensor_mul(prd[:], xns[d][:], ytos[d][:])
                nc.tensor.matmul(p_dd[:], ones_bf[:], prd[:],
                                 start=(d == 0), stop=(d == ND - 1),
                                 skip_group_check=True)
            # dk0 = sum_i p_dd_i * ryo_i / tau   (p_dd already has 1/||x||)
            v1 = tmp.tile([1, 512], F32, tag="v", name="v1")
            nc.vector.tensor_mul(v1[:], p_dd[:], ryo[:])
            v3 = tmp.tile([1, 512], F32, tag="v", name="v3")
            nc.vector.tensor_scalar(v3[:], v1[:], 1.0 / TAU, None,
                                    ALU.mult, ALU.add,
                                    accum_out=dk_rk[:, 0:1])

            for jb in range(16, NJB):
                emit_main_block(jb)
            for jb in range(NJB - RS_LAG, NJB):
                emit_rowmm(jb)

            # ---- row term: dk_rk[1] = sum_i ln(row_denom_i) ----
            rdv = tmp.tile([1, 512], F32, tag="v", name="rdv")
            nc.vector.tensor_scalar_add(rdv[:], p_row[:], EXTRA)
            rlnv = tmp.tile([1, 512], F32, tag="v", name="rlnv")
            nc.scalar.activation(rlnv[:], rdv[:], AF.Ln,
                                 accum_out=dk_rk[:, 1:2])

            # ---- single merged AllReduce: all col partials + scalars ----
            ar_in = dr.tile([4104], F32, name="ar_in")
            ar_out = dr.tile([4104], F32, name="ar_out")
            nc.sync.dma_start(ar_in[0:4096], colpart[:])
            nc.sync.dma_start(ar_in[4096:4104], dk_rk[:])
            nc.gpsimd.collective_compute(
                "AllReduce", ALU.add, replica_groups=rg,
                ins=[ar_in.opt()], outs=[ar_out.opt()])

            # ---- col term + final scalar (replicated on every core) ----
            csum = tmp.tile([128, 32], F32, tag="w", name="csum")
            nc.sync.dma_start(csum[:], ar_out[0:4096])
            sc2 = tmp.tile([1, 2], F32, tag="s2", name="sc2", bufs=1)
            nc.sync.dma_start(sc2[:], ar_out[4096:4098])
            cd = tmp.tile([128, 32], F32, tag="w", name="cd")
            nc.vector.tensor_scalar_add(cd[:], csum[:], EXTRA)
            cln = tmp.tile([128, 32], F32, tag="w", name="cln")
            cacc = res.tile([128, 1], F32, name="cacc")
            nc.scalar.activation(cln[:], cd[:], AF.Ln,
                                 accum_out=cacc[:, 0:1])
            p_s = pa.tile([1, 1], F32, tag="pa", name="p_s")
            nc.tensor.matmul(p_s[:], ones_f[:], cacc[:, 0:1],
                             start=True, stop=True, skip_group_check=True)

            f1 = res.tile([1, 1], F32, name="f1")
            nc.vector.tensor_scalar_mul(f1[:], sc2[:, 0:1], 2.0)
            f2 = res.tile([1, 1], F32, name="f2")
            nc.vector.tensor_sub(f2[:], f1[:], sc2[:, 1:2])
            f3 = res.tile([1, 1], F32, name="f3")
            nc.vector.tensor_sub(f3[:], f2[:], p_s[:])
            fl = res.tile([1, 1], F32, name="fl")
            nc.vector.tensor_scalar_mul(fl[:], f3[:], COEF)
            nc.sync.dma_start(loss_out[:, :], fl[:])

    nc.compile()
    return nc


def get_nc():
    if "nc" not in _cache:
        _cache["nc"] = _build()
    return _cache["nc"]


def make_in_maps(x: np.ndarray, y: np.ndarray):
    xb = x.astype(ml_dtypes.bfloat16)
    yb = y.astype(ml_dtypes.bfloat16)
    xT = np.ascontiguousarray(xb.T)
    yT = np.ascontiguousarray(yb.T)
    in_maps = []
    for k in range(N_CORES):
        in_maps.append({
            "xT": np.ascontiguousarray(xT[:, k * BL:(k + 1) * BL]),
            "yT": yT,
            "yTown": np.ascontiguousarray(yT[:, k * BL:(k + 1) * BL]),
        })
    return in_maps


def kernel(x: np.ndarray, y: np.ndarray) -> np.ndarray:
    nc = get_nc()
    in_maps = make_in_maps(np.asarray(x), np.asarray(y))
    res = run_bass_kernel_spmd(nc, in_maps, core_ids=list(range(N_CORES)))
    loss = res.results[0]["loss"]
    return np.asarray(loss, dtype=np.float32).reshape(())
